# revision 69
# baseline (speedup 1.0000x reference)
"""Trainium2 Bass kernel for nn_DiTBlock (B=4,N=1024,C=1024,H=16).

8-way SPMD: core i handles batch i//2; the two cores of a batch split rows
interleaved (core rh takes permuted rows rh::2). Host permutes rows so
unmasked (key) rows come first; keys are compacted to KPAD=640 columns
(masked tail keys contribute exp(-1e4)=0, so any pad >= #unmasked is exact).

All linear layers and attention*V run in fp8e4 with
MatmulPerfMode.DoubleRow (K=256 per instruction); weights are host-scaled
by 16 (avoids fp8 subnormals) and descaled at PSUM->SBUF conversion
points. fc1/fc2 use hi/lo split weights plus an ln3 low-order pass (the
lo residuals live in fp8 subnormals, so they accumulate at matched scale)
to stay inside the 2e-2 error gate. Scores are bf16 per head (K=64, head
slots at partition 0/64); softmax exp runs on the scalar engine with the
additive mask as per-partition bias (scale 2^-11 folds 1/sqrt(d) and the
16x16 weight scaling). The denominator rides as a ones-column in V;
attention*V is computed row-major (queries on partitions) so the divide
is one gpsimd normalize_recip, then transposed back feature-major via PE.
The attention loop is software-pipelined: S/exp of head-pair hp issues
ahead of AV/renorm of hp-1 so the scalar engine stays saturated. MLP
weights prefetch during CA attention where DMA is otherwise idle.
"""
import numpy as np
from contextlib import ExitStack

import concourse.bass as bass
import concourse.bacc as bacc
import concourse.mybir as mybir
import concourse.tile as tile
from concourse.bass_utils import run_bass_kernel_spmd
from concourse.masks import make_identity

F32 = mybir.dt.float32
BF16 = mybir.dt.bfloat16
FP8 = mybir.dt.float8e4
AF = mybir.ActivationFunctionType
ALU = mybir.AluOpType
DR = mybir.MatmulPerfMode.DoubleRow

B, N, C, H, D = 4, 1024, 1024, 16, 64
HID = 4 * C
R = 512            # own rows per core
KOWN = 320         # own keys per core
KPAD = 2 * KOWN    # padded key count (640 = 5 chunks of 128)
NKY = KPAD // 128  # 5
EPS = 1e-6
WS = 16.0          # weight scale for fp8
IWS = 1.0 / WS

_cache = {}


def build_program(nb=False):
    # nb: True if any proj/caproj/fc2 bias is nonzero (adds bias-row ops)
    nc = bacc.Bacc(None, target_bir_lowering=False)

    x_own = nc.dram_tensor("x_own", [R, C], F32, kind="ExternalInput")
    x_othk = nc.dram_tensor("x_othk", [KOWN, C], F32, kind="ExternalInput")
    cT_d = nc.dram_tensor("cT", [C, KPAD], FP8, kind="ExternalInput")
    mask_d = nc.dram_tensor("mask", [128, NKY], F32, kind="ExternalInput")
    wq_d = nc.dram_tensor("wq", [C, C], FP8, kind="ExternalInput")
    wk_d = nc.dram_tensor("wk", [C, C], FP8, kind="ExternalInput")
    wv_d = nc.dram_tensor("wv", [C, C], FP8, kind="ExternalInput")
    wproj_d = nc.dram_tensor("wproj", [C, C], FP8, kind="ExternalInput")
    wcaq_d = nc.dram_tensor("wcaq", [C, C], FP8, kind="ExternalInput")
    wcak_d = nc.dram_tensor("wcak", [C, C], FP8, kind="ExternalInput")
    wcav_d = nc.dram_tensor("wcav", [C, C], FP8, kind="ExternalInput")
    wcaproj_d = nc.dram_tensor("wcaproj", [C, C], FP8, kind="ExternalInput")
    wfc1_d = nc.dram_tensor("wfc1", [C, HID], FP8, kind="ExternalInput")
    wfc2_d = nc.dram_tensor("wfc2", [HID, C], FP8, kind="ExternalInput")
    wfc1l_d = nc.dram_tensor("wfc1l", [C, HID], FP8, kind="ExternalInput")
    wfc2l_d = nc.dram_tensor("wfc2l", [HID, C], FP8, kind="ExternalInput")
    bq_d = nc.dram_tensor("bq16", [128, 8], F32, kind="ExternalInput")
    bk_d = nc.dram_tensor("bk16", [128, 8], F32, kind="ExternalInput")
    bcaq_d = nc.dram_tensor("bcaq16", [128, 8], F32, kind="ExternalInput")
    bcak_d = nc.dram_tensor("bcak16", [128, 8], F32, kind="ExternalInput")
    bfc1_d = nc.dram_tensor("bfc1", [128, 32], F32, kind="ExternalInput")
    bpj_d = nc.dram_tensor("bpj", [1, 3 * C], F32, kind="ExternalInput")
    y_d = nc.dram_tensor("y", [R, C], F32, kind="ExternalOutput")

    with tile.TileContext(nc) as tc, ExitStack() as ctx:
        misc = ctx.enter_context(tc.tile_pool(name="misc", bufs=1))
        pxp = ctx.enter_context(tc.tile_pool(name="pxp", bufs=1))
        lnrmp = ctx.enter_context(tc.tile_pool(name="lnrmp", bufs=4))
        statp = ctx.enter_context(tc.tile_pool(name="statp", bufs=4))
        smp = ctx.enter_context(tc.tile_pool(name="smp", bufs=4))
        finp = ctx.enter_context(tc.tile_pool(name="finp", bufs=2))
        wcolp = ctx.enter_context(tc.tile_pool(name="wcolp", bufs=4))

        ident32 = misc.tile([128, 128], F32)
        make_identity(nc, ident32)
        ident16 = misc.tile([128, 128], BF16)
        nc.vector.tensor_copy(ident16, ident32)
        ones64 = misc.tile([1, 64], BF16)
        nc.vector.memset(ones64, 1.0)
        eps_b = misc.tile([128, 1], F32)
        nc.gpsimd.memset(eps_b, EPS)
        msk = misc.tile([128, NKY], F32)
        nc.sync.dma_start(out=msk, in_=mask_d[:, :])
        # Schraudolph fast-exp encoding of the mask bias for DVE-exp tiles:
        # 2^(log2e*(2^-11*S + msk)) via int32 convert + bitcast
        FEA = 1.4426950408889634 * (1 << 23)
        fmsk = misc.tile([128, NKY], F32)
        nc.vector.tensor_scalar(fmsk, msk, FEA,
                                float(127 * (1 << 23) - 366393.0),
                                ALU.mult, ALU.add)
        bq16 = misc.tile([128, 8], F32)
        nc.sync.dma_start(out=bq16, in_=bq_d[:, :])
        bk16 = misc.tile([128, 8], F32)
        nc.sync.dma_start(out=bk16, in_=bk_d[:, :])
        bcaq16 = misc.tile([128, 8], F32)
        nc.sync.dma_start(out=bcaq16, in_=bcaq_d[:, :])
        bcak16 = misc.tile([128, 8], F32)
        nc.sync.dma_start(out=bcak16, in_=bcak_d[:, :])
        bfc1 = misc.tile([128, 32], F32)
        nc.sync.dma_start(out=bfc1, in_=bfc1_d[:, :])

        # proj/caproj/fc2 bias rows broadcast across partitions
        bpj = None
        if nb:
            bpj = misc.tile([128, 3, C], F32)
            s = bpj_d[0:1, :]
            ap = bass.AP(tensor=s.tensor, offset=s.offset,
                         ap=[[0, 128], [1, 3 * C]])
            nc.gpsimd.dma_start(out=bpj, in_=ap)

        wmlp = ctx.enter_context(tc.tile_pool(name="wmlp", bufs=1))

        x_sb = pxp.tile([128, 4, C], F32, tag="x")
        for rt in range(4):
            for hc in range(2):
                nc.sync.dma_start(
                    out=x_sb[:, rt, hc * 512:(hc + 1) * 512],
                    in_=x_own[rt * 128:(rt + 1) * 128,
                              hc * 512:(hc + 1) * 512])

        def ln_stats(src_ap, P=128):
            st = statp.tile([128, 2, 6], F32, tag="st", name="st")
            for sg in range(2):
                nc.vector.bn_stats(out=st[0:P, sg, :],
                                   in_=src_ap[0:P, sg * 512:(sg + 1) * 512])
            if P < 128:
                nc.gpsimd.memset(st[P:128, :, :], 0.0)
            return st

        def ln_finish(sts, src_aps, Ps=None, engs=None):
            n = len(src_aps)
            if Ps is None:
                Ps = [128] * n
            if engs is None:
                engs = [nc.vector] * n
            mvs = statp.tile([128, 4, 2], F32, tag="mvs", name="mvs")
            for g, st in enumerate(sts):
                nc.vector.bn_aggr(out=mvs[:, g, :], in_=st)
            # rstd = rsqrt(var+eps), batched (bit-trick seed + 2 Newton steps)
            ve = statp.tile([128, 4], F32, tag="ve", name="ve")
            nc.vector.tensor_scalar_add(ve[:, :n], mvs[:, :n, 1], eps_b)
            iv = statp.tile([128, 4], mybir.dt.int32, tag="iv", name="iv")
            nc.vector.tensor_scalar(iv[:, :n], ve[:, :n].bitcast(mybir.dt.int32), 1,
                                    None, ALU.arith_shift_right)
            nc.vector.tensor_scalar(iv[:, :n], iv[:, :n], -1, 0x5F3759DF,
                                    ALU.mult, ALU.add)
            y = iv.bitcast(F32)
            u = statp.tile([128, 4], F32, tag="u", name="u")
            for _ in range(2):
                nc.vector.tensor_tensor(u[:, :n], y[:, :n], y[:, :n], ALU.mult)
                nc.vector.tensor_tensor(u[:, :n], u[:, :n], ve[:, :n], ALU.mult)
                nc.vector.tensor_scalar(u[:, :n], u[:, :n], -0.5, 1.5, ALU.mult, ALU.add)
                nc.vector.tensor_tensor(y[:, :n], y[:, :n], u[:, :n], ALU.mult)
            outs = []
            for g, src_ap in enumerate(src_aps):
                P = Ps[g]
                t = lnrmp.tile([128, C], BF16, tag="lnrm", name="lnt")
                engs[g].tensor_scalar(t[0:P, :], src_ap[0:P, :], mvs[0:P, g, 0:1],
                                      y[0:P, g:g + 1], ALU.subtract, ALU.mult)
                outs.append(t)
            return outs

        def ln_group(src_aps, Ps=None, engs=None):
            if Ps is None:
                Ps = [128] * len(src_aps)
            return ln_finish([ln_stats(a, P) for a, P in zip(src_aps, Ps)],
                             src_aps, Ps, engs)

        def transpose_group(srcs, Ps, dst_fn, psT, lo_fn=None):
            # srcs: row tiles [P_i, C]; writes dst_fn(ct) [128, sum(Ps)] fp8;
            # lo_fn(ct): optional fp8 residual (tp - hi), exact via subnormals
            W = sum(Ps)
            for ct in range(8):
                tp = psT.tile([128, 512], BF16, tag="T", name="tp")
                col = 0
                for src, P in zip(srcs, Ps):
                    nc.tensor.transpose(tp[:, col:col + P],
                                        src[0:P, ct * 128:(ct + 1) * 128],
                                        ident16[0:P, 0:P])
                    col += P
                nc.scalar.copy(dst_fn(ct), tp[:, 0:W])
                if lo_fn is not None:
                    nc.vector.tensor_tensor(lo_fn(ct), tp[:, 0:W], dst_fn(ct),
                                            ALU.subtract)

        def colblock(w_handle, o0, width=256):
            wc = wcolp.tile([128, 8, width], FP8, tag="wcol", name="wc")
            nc.sync.dma_start(out=wc,
                              in_=w_handle[:, o0:o0 + width].rearrange(
                                  "(kt p) o -> p kt o", p=128))
            return wc

        def rowblock(w_handle, nkt, pool, tag="wrow"):
            # two 512-col tiles so a [:, pair, :] slice stays contiguous
            # (fp8 DR moving operands >512 elements must be contiguous)
            ws = []
            for hc in range(2):
                wr = pool.tile([128, nkt, 512], FP8, tag=f"{tag}{hc}",
                               name=f"{tag}{hc}")
                nc.sync.dma_start(
                    out=wr,
                    in_=w_handle[:, hc * 512:(hc + 1) * 512].rearrange(
                        "(kt p) o -> p kt o", p=128))
                ws.append(wr)
            return ws

        # ---------------- attention (shared for sa/ca) ----------------
        nrp = ctx.enter_context(tc.tile_pool(name="nrp", bufs=3))

        def attention(qTt, kTt, Vt, attnTt, psS, psA, psTat, expp):
            # qTt [128, 8ot, 512] bf16 (ot = head pair, slots at 0/64);
            # kTt [128, 8ot, KPAD] bf16; Vt [128, NKY, 16, 65] fp8;
            # out attnTt [128, 8, 512] fp8.
            # S computed per head-quad into one [128, 2048] psum (one exp);
            # AV is row-major (queries on partitions) so the softmax denom
            # is per-partition: normalize_recip on gpsimd does the divide;
            # the normalized rows transpose back to feature-major via PE.
            def s_exp(hp):
                exA = expp.tile([128, 2, 2, 512], FP8, tag="ex", name="exA")
                exB = expp.tile([128, 2, 2, 512], FP8, tag="ex", name="exB")
                exC = expp.tile([128, 2, 512], FP8, tag="ex4", name="exC", bufs=2)
                exd = {0: exA[:, 0, :, :], 1: exA[:, 1, :, :],
                       2: exB[:, 0, :, :], 3: exB[:, 1, :, :],
                       4: exC[:, :, :]}
                for ky in range(NKY):
                    ps = psS.tile([128, 1024], F32, tag="S", name="ps")
                    for s2 in range(2):
                        h = 2 * hp + s2
                        sl = slice((h % 2) * 64, (h % 2) * 64 + 64)
                        nc.tensor.matmul(
                            ps[:, s2 * 512:(s2 + 1) * 512],
                            kTt[sl, h // 2, ky * 128:(ky + 1) * 128],
                            qTt[sl, h // 2, :],
                            start=True, stop=True)
                    if ky == 0:
                        iv = smp.tile([128, 1024], mybir.dt.int32, tag="fex",
                                      name="iv", bufs=2)
                        nc.vector.tensor_scalar(iv, ps,
                                                FEA * 2.0 ** -11,
                                                fmsk[:, 0:1],
                                                ALU.mult, ALU.add)
                        nc.vector.tensor_scalar(exd[0], iv.bitcast(F32),
                                                0.0, None, ALU.max)
                    else:
                        nc.scalar.activation(exd[ky], ps, AF.Exp,
                                             bias=msk[:, ky:ky + 1],
                                             scale=2.0 ** -11)
                return exA, exB, exC

            def av(hp, exs):
                exA, exB, exC = exs
                for s2 in range(2):
                    h = 2 * hp + s2
                    pa4 = psA.tile([128, 4, 65], F32, tag="A", name="pa4")
                    for qc in range(4):
                        qsl = slice(qc * 128, (qc + 1) * 128)
                        nc.tensor.matmul(pa4[:, qc, :], exA[:, :, s2, qsl],
                                         Vt[:, 0:2, h, :],
                                         start=True, stop=False, perf_mode=DR)
                        nc.tensor.matmul(pa4[:, qc, :], exB[:, :, s2, qsl],
                                         Vt[:, 2:4, h, :],
                                         start=False, stop=False, perf_mode=DR)
                        nc.tensor.matmul(pa4[:, qc, :], exC[:, s2, qsl],
                                         Vt[:, 4, h, :],
                                         start=False, stop=True)
                    aw = smp.tile([128, 4, 65], F32, tag="aw", name="aw")
                    nc.vector.tensor_scalar(aw, pa4, IWS, None, ALU.mult)
                    nrow = nrp.tile([128, 4, 64], BF16, tag="nrow", name="nrow")
                    for qc in range(4):
                        nc.gpsimd.normalize_recip(nrow[:, qc, :],
                                                  aw[:, qc, 0:64],
                                                  aw[:, qc, 64:65])
                    tp = psTat.tile([64, 512], BF16, tag="Tat", name="tpa")
                    for qc in range(4):
                        nc.tensor.transpose(tp[:, qc * 128:(qc + 1) * 128],
                                            nrow[:, qc, :], ident16)
                    dst = attnTt[(h % 2) * 64:(h % 2) * 64 + 64, h // 2, :]
                    nc.vector.tensor_copy(dst, tp)

            prev = None
            for hp in range(8):
                exs = s_exp(hp)
                if prev is not None:
                    av(prev[0], prev[1])
                prev = (hp, exs)
            av(prev[0], prev[1])

        # ---------------- Phase A: LN1 + transposes ----------------
        with tc.tile_pool(name="qTp", bufs=1) as qTp, \
             tc.tile_pool(name="kTp", bufs=1) as kTp, \
             tc.tile_pool(name="Vp", bufs=1) as Vp, \
             tc.tile_pool(name="cavp", bufs=1) as cavp, \
             tc.tile_pool(name="cTp", bufs=1) as cTp, \
             tc.tile_pool(name="attnTp", bufs=1) as attnTp, \
             tc.tile_pool(name="wrowp", bufs=1) as wrowp:

            lnTq = ctx2 = None
            lnT_ctx = tc.tile_pool(name="lnTq", bufs=1)
            lnTq = lnT_ctx.__enter__()
            xop_ctx = tc.tile_pool(name="xop", bufs=3)
            xop = xop_ctx.__enter__()
            ln1o = lnTq.tile([128, 8, 512], FP8, tag="ln1o")
            ln1x = lnTq.tile([128, 8, KOWN], FP8, tag="ln1x")

            with tc.tile_pool(name="psT", bufs=2, space="PSUM") as psT, \
                 tc.tile_pool(name="psL", bufs=2, space="PSUM") as psL, \
                 tc.tile_pool(name="psX", bufs=1, space="PSUM") as psX, \
                 tc.tile_pool(name="psK", bufs=1, space="PSUM") as psK:

                srcs = ln_group([x_sb[:, rt, :] for rt in range(4)])
                transpose_group(srcs, [128] * 4, lambda ct: ln1o[:, ct, :], psT)

                xo_tiles = []
                oPs = [128, 128, 64]
                for i, P in enumerate(oPs):
                    xo = xop.tile([128, C], F32, tag="xoth", name="xo")
                    for hc in range(2):
                        nc.sync.dma_start(
                            out=xo[0:P, hc * 512:(hc + 1) * 512],
                            in_=x_othk[i * 128:i * 128 + P,
                                       hc * 512:(hc + 1) * 512])
                    xo_tiles.append(xo)
                osrcs = ln_group(xo_tiles, oPs, engs=[nc.gpsimd] * 3)
                transpose_group(osrcs, oPs, lambda ct: ln1x[:, ct, :], psT)

                # ---------------- Phase B: sa qT / kT / V ----------------
                qT = qTp.tile([128, 8, 512], BF16, tag="qT")
                kT = kTp.tile([128, 8, KPAD], BF16, tag="kT")
                for og in range(4):
                    wc = colblock(wq_d, og * 256)
                    for oi in range(2):
                        ot = og * 2 + oi
                        pq = psL.tile([128, 512], F32, tag="L", name="pq")
                        for t in range(4):
                            nc.tensor.matmul(pq, wc[:, 2 * t:2 * t + 2,
                                                    oi * 128:(oi + 1) * 128],
                                             ln1o[:, 2 * t:2 * t + 2, :],
                                             start=(t == 0), stop=(t == 3),
                                             perf_mode=DR)
                        nc.vector.tensor_scalar(qT[:, ot, :], pq,
                                                bq16[:, ot:ot + 1], None, ALU.add)
                for og in range(4):
                    wc = colblock(wk_d, og * 256)
                    for oi in range(2):
                        ot = og * 2 + oi
                        pk = psK.tile([128, 1024], F32, tag="K", name="pk")
                        for t in range(4):
                            nc.tensor.matmul(pk[:, 0:512],
                                             wc[:, 2 * t:2 * t + 2,
                                                oi * 128:(oi + 1) * 128],
                                             ln1o[:, 2 * t:2 * t + 2, :],
                                             start=(t == 0), stop=(t == 3),
                                             perf_mode=DR)
                        for t in range(4):
                            nc.tensor.matmul(pk[:, 512:512 + KOWN],
                                             wc[:, 2 * t:2 * t + 2,
                                                oi * 128:(oi + 1) * 128],
                                             ln1x[:, 2 * t:2 * t + 2, :],
                                             start=(t == 0), stop=(t == 3),
                                             perf_mode=DR)
                        nc.vector.tensor_scalar(
                            kT[:, ot, :].rearrange("p (a b) -> p a b", a=2),
                            pk.rearrange("p (a b) -> p a b", a=2)[:, :, 0:KOWN],
                            bk16[:, ot:ot + 1], None, ALU.add)

                V = Vp.tile([128, NKY, 16, 65], FP8, tag="V")
                nc.vector.memset(V[:, :, :, 64:65], 1.0)
                wv_sb = rowblock(wv_d, 8, wrowp, tag="wv")
                # key chunks: [own 0:128, own 128:256, own 256:320 + oth 0:64,
                #              oth 64:192, oth 192:320]
                vchunks = [[(ln1o, 0, 128, 0)],
                           [(ln1o, 128, 128, 0)],
                           [(ln1o, 256, 64, 0), (ln1x, 0, 64, 64)],
                           [(ln1x, 64, 128, 0)],
                           [(ln1x, 192, 128, 0)]]

                def build_V(Vt, src_sb, chunks, Vpsum, Vpsx=None):
                    # parts with p0 > 0 cannot be matmul outputs (psum base
                    # partition must be 0): compute at base 0 and DMA-shift.
                    for ks, parts in enumerate(chunks):
                        for vc in range(2):
                            pv = Vpsum.tile([128, 512], F32, tag="L", name="pv")
                            for (srcT, c0, cn, p0) in parts:
                                if p0 == 0:
                                    for t in range(4):
                                        nc.tensor.matmul(
                                            pv[0:cn, :],
                                            srcT[:, 2 * t:2 * t + 2, c0:c0 + cn],
                                            src_sb[vc][:, 2 * t:2 * t + 2, :],
                                            start=(t == 0), stop=(t == 3),
                                            perf_mode=DR)
                                else:
                                    pvx = Vpsx.tile([64, 512], F32, tag="Lx",
                                                    name="pvx")
                                    for t in range(4):
                                        nc.tensor.matmul(
                                            pvx,
                                            srcT[:, 2 * t:2 * t + 2, c0:c0 + cn],
                                            src_sb[vc][:, 2 * t:2 * t + 2, :],
                                            start=(t == 0), stop=(t == 3),
                                            perf_mode=DR)
                                    scr = smp.tile([64, 8, 64], FP8, tag="vscr",
                                                   name="vscr", bufs=2)
                                    nc.vector.tensor_scalar(
                                        scr, pvx.rearrange("p (h d) -> p h d",
                                                           h=8),
                                        IWS, None, ALU.mult)
                                    nc.gpsimd.dma_start(
                                        out=Vt[64:128, ks,
                                               vc * 8:(vc + 1) * 8, 0:64],
                                        in_=scr)
                            P0 = max(cn for (_, _, cn, p0) in parts
                                     if p0 == 0)
                            nc.scalar.activation(
                                Vt[0:P0, ks, vc * 8:(vc + 1) * 8, 0:64],
                                pv[0:P0, :].rearrange("p (h d) -> p h d", h=8),
                                AF.Identity, scale=IWS)

                build_V(V, wv_sb, vchunks, psL, psX)

            xop_ctx.__exit__(None, None, None)
            lnT_ctx.__exit__(None, None, None)

            # ---------------- Phase C: SA attention ----------------
            attnT = attnTp.tile([128, 8, 512], FP8, tag="attnT")
            cT_sb = cTp.tile([128, 8, KPAD], FP8, tag="cT")
            nc.sync.dma_start(out=cT_sb,
                              in_=cT_d[:, :].rearrange("(kt p) k -> p kt k", p=128))
            with tc.tile_pool(name="psS", bufs=2, space="PSUM") as psS, \
                 tc.tile_pool(name="psA", bufs=2, space="PSUM") as psA, \
                 tc.tile_pool(name="psR", bufs=2, space="PSUM") as psR, \
                 tc.tile_pool(name="expp", bufs=4) as expp:
                attention(qT, kT, V, attnT, psS, psA, psR, expp)  # psR = psTat

            # ------- Phase D+E: ca k/v interleaved with sa proj/LN2/caq -----
            cakT = kTp.tile([128, 8, KPAD], BF16, tag="kT", name="cakT")
            caV = cavp.tile([128, NKY, 16, 65], FP8, tag="caV")
            wpj_sb = rowblock(wproj_d, 8, wrowp, tag="wpj")
            caqT = qTp.tile([128, 8, 512], BF16, tag="qT", name="caqT")

            def cak_og(og, psK2):
                wc = colblock(wcak_d, og * 256)
                for oi in range(2):
                    ot = og * 2 + oi
                    pk = psK2.tile([128, 1024], F32, tag="K2", name="pk2")
                    for c0 in (0, 256, 512):
                        cn = 128 if c0 == 512 else 256
                        for t in range(4):
                            nc.tensor.matmul(pk[:, c0:c0 + cn],
                                             wc[:, 2 * t:2 * t + 2,
                                                oi * 128:(oi + 1) * 128],
                                             cT_sb[:, 2 * t:2 * t + 2,
                                                   c0:c0 + cn],
                                             start=(t == 0), stop=(t == 3),
                                             perf_mode=DR)
                    nc.scalar.activation(cakT[:, ot, :], pk[:, 0:KPAD],
                                             AF.Identity,
                                             bias=bcak16[:, ot:ot + 1])

            def proj_rt(attnTt, w_sb, bias_idx, rt, psP):
                for oc in range(2):
                    pp = psP.tile([128, 512], F32, tag="P", name="pp")
                    for t in range(4):
                        nc.tensor.matmul(pp,
                                         attnTt[:, 2 * t:2 * t + 2,
                                                rt * 128:(rt + 1) * 128],
                                         w_sb[oc][:, 2 * t:2 * t + 2, :],
                                         start=(t == 0), stop=(t == 3),
                                         perf_mode=DR)
                    xsl = x_sb[:, rt, oc * 512:(oc + 1) * 512]
                    nc.vector.scalar_tensor_tensor(xsl, pp, IWS, xsl,
                                                   ALU.mult, ALU.add)
                    if nb:
                        nc.gpsimd.tensor_tensor(
                            xsl, xsl,
                            bpj[:, bias_idx, oc * 512:(oc + 1) * 512], ALU.add)
                return ln_stats(x_sb[:, rt, :])

            def proj_residual(attnTt, w_sb, bias_idx, psP):
                sts = [proj_rt(attnTt, w_sb, bias_idx, rt, psP)
                       for rt in range(4)]
                return ln_finish(sts, [x_sb[:, r, :] for r in range(4)],
                                 engs=[nc.gpsimd, nc.vector,
                                       nc.gpsimd, nc.vector])

            with tc.tile_pool(name="lnT2p", bufs=1) as lnT2p:
                with tc.tile_pool(name="psK2", bufs=1, space="PSUM") as psK2, \
                     tc.tile_pool(name="psV2", bufs=2, space="PSUM") as psV2, \
                     tc.tile_pool(name="psP", bufs=2, space="PSUM") as psP, \
                     tc.tile_pool(name="psT2", bufs=2, space="PSUM") as psT2:
                    sts = []
                    for rt in range(4):
                        cak_og(rt, psK2)
                        sts.append(proj_rt(attnT, wpj_sb, 0, rt, psP))
                    lns2 = ln_finish(sts, [x_sb[:, r, :] for r in range(4)],
                                     engs=[nc.gpsimd, nc.vector,
                                           nc.gpsimd, nc.vector])
                    nc.vector.memset(caV[:, :, :, 64:65], 1.0)
                    wcav_sb = rowblock(wcav_d, 8, wrowp, tag="wv")
                    cchunks = [[(cT_sb, ks * 128, 128, 0)] for ks in range(NKY)]
                    build_V(caV, wcav_sb, cchunks, psV2)
                    ln2T = lnT2p.tile([128, 8, 512], FP8, tag="ln2T")
                    transpose_group(lns2, [128] * 4,
                                    lambda ct: ln2T[:, ct, :], psT2)
                    for og in range(4):
                        wc = colblock(wcaq_d, og * 256)
                        for oi in range(2):
                            ot = og * 2 + oi
                            pq = psP.tile([128, 512], F32, tag="P", name="pq2")
                            for t in range(4):
                                nc.tensor.matmul(pq, wc[:, 2 * t:2 * t + 2,
                                                        oi * 128:(oi + 1) * 128],
                                                 ln2T[:, 2 * t:2 * t + 2, :],
                                                 start=(t == 0), stop=(t == 3),
                                                 perf_mode=DR)
                            nc.scalar.activation(caqT[:, ot, :], pq, AF.Identity,
                                                 bias=bcaq16[:, ot:ot + 1])

            # ---- prefetch MLP weights (DMA idles during CA attention) ----
            wf1 = wmlp.tile([128, 8, HID], FP8, tag="wf1")
            wf1l = wmlp.tile([128, 8, HID], FP8, tag="wf1l")
            for kt in range(8):
                nc.sync.dma_start(out=wf1[:, kt, :],
                                  in_=wfc1_d[kt * 128:(kt + 1) * 128, :])
                nc.sync.dma_start(out=wf1l[:, kt, :],
                                  in_=wfc1l_d[kt * 128:(kt + 1) * 128, :])

            # ---------------- Phase F: CA attention ----------------
            caattnT = attnTp.tile([128, 8, 512], FP8, tag="attnT",
                                  name="caattnT")
            with tc.tile_pool(name="psS2", bufs=2, space="PSUM") as psS2, \
                 tc.tile_pool(name="psA2", bufs=2, space="PSUM") as psA2, \
                 tc.tile_pool(name="psR2", bufs=2, space="PSUM") as psR2, \
                 tc.tile_pool(name="expp2", bufs=4) as expp2:
                attention(caqT, cakT, caV, caattnT, psS2, psA2, psR2, expp2)

            # ---------------- Phase G: ca proj + resid + LN3 -----------
            wcpj_sb = rowblock(wcaproj_d, 8, wrowp, tag="wpj", )
            with tc.tile_pool(name="psP3", bufs=2, space="PSUM") as psP3:
                lns3 = proj_residual(caattnT, wcpj_sb, 1, psP3)

        # ---------------- Phase H: MLP ----------------
        with tc.tile_pool(name="mlpp", bufs=1) as mlpp, \
             tc.tile_pool(name="psM", bufs=2, space="PSUM") as psM, \
             tc.tile_pool(name="psT3", bufs=2, space="PSUM") as psT3:
            ln3T = mlpp.tile([128, 8, 512], FP8, tag="lnT3")
            ln3L = mlpp.tile([128, 8, 512], FP8, tag="lnT3L")
            transpose_group(lns3, [128] * 4, lambda ct: ln3T[:, ct, :], psT3,
                            lo_fn=lambda ct: ln3L[:, ct, :])

            wf2 = rowblock(wfc2_d, 32, mlpp, tag="wfc2")
            wf2l = rowblock(wfc2l_d, 32, mlpp, tag="wfc2l")
            h1gT = mlpp.tile([128, 32, 512], FP8, tag="h1gT")
            for og in range(16):
                pf = psM.tile([128, 1024], F32, tag="M", name="pf")
                for oi in range(2):
                    ot = og * 2 + oi
                    po = pf[:, oi * 512:(oi + 1) * 512]
                    wsl = wf1[:, :, ot * 128:(ot + 1) * 128]
                    wsll = wf1l[:, :, ot * 128:(ot + 1) * 128]
                    for t in range(4):
                        nc.tensor.matmul(po, wsl[:, 2 * t:2 * t + 2, :],
                                         ln3T[:, 2 * t:2 * t + 2, :],
                                         start=(t == 0), stop=False,
                                         perf_mode=DR)
                    for t in range(4):
                        nc.tensor.matmul(po, wsl[:, 2 * t:2 * t + 2, :],
                                         ln3L[:, 2 * t:2 * t + 2, :],
                                         start=False, stop=False,
                                         perf_mode=DR)
                    for t in range(4):
                        nc.tensor.matmul(po, wsll[:, 2 * t:2 * t + 2, :],
                                         ln3T[:, 2 * t:2 * t + 2, :],
                                         start=False, stop=(t == 3),
                                         perf_mode=DR)
                if nb:
                    for oi in range(2):
                        ot = og * 2 + oi
                        nc.scalar.activation(h1gT[:, ot, :],
                                             pf[:, oi * 512:(oi + 1) * 512],
                                             AF.Gelu, bias=bfc1[:, ot:ot + 1],
                                             scale=IWS)
                else:
                    nc.scalar.activation(
                        h1gT[:, 2 * og:2 * og + 2, :].rearrange(
                            "p a b -> p (a b)"),
                        pf, AF.Gelu, scale=IWS)

            for rt in range(4):
                for oc in range(2):
                    pm = psM.tile([128, 512], F32, tag="M", name="pm")
                    for t in range(16):
                        nc.tensor.matmul(pm,
                                         h1gT[:, 2 * t:2 * t + 2,
                                              rt * 128:(rt + 1) * 128],
                                         wf2[oc][:, 2 * t:2 * t + 2, :],
                                         start=(t == 0), stop=False,
                                         perf_mode=DR)
                    for t in range(16):
                        nc.tensor.matmul(pm,
                                         h1gT[:, 2 * t:2 * t + 2,
                                              rt * 128:(rt + 1) * 128],
                                         wf2l[oc][:, 2 * t:2 * t + 2, :],
                                         start=False, stop=(t == 15),
                                         perf_mode=DR)
                    fin = finp.tile([128, 512], F32, tag="fin", name="fin")
                    nc.vector.scalar_tensor_tensor(
                        fin, pm, IWS, x_sb[:, rt, oc * 512:(oc + 1) * 512],
                        ALU.mult, ALU.add)
                    if nb:
                        nc.gpsimd.tensor_tensor(
                            fin, fin, bpj[:, 2, oc * 512:(oc + 1) * 512], ALU.add)
                    nc.sync.dma_start(
                        out=y_d[rt * 128:(rt + 1) * 128, oc * 512:(oc + 1) * 512],
                        in_=fin)

    nc.finalize()
    return nc


def _fp8(a):
    import ml_dtypes
    return np.clip(a, -240.0, 240.0).astype(ml_dtypes.float8_e4m3)


def _prep_inputs(i, x, c, mask, sa_qkv_w, sa_qkv_b, sa_proj_w, sa_proj_b,
                 ca_q_w, ca_q_b, ca_k_w, ca_k_b, ca_v_w, ca_v_b,
                 ca_proj_w, ca_proj_b, fc1_w, fc1_b, fc2_w, fc2_b):
    f = np.float32
    b, rh = i // 2, i % 2
    perm = np.argsort(mask[b] != 1, kind="stable")
    perm_own = perm[rh::2]
    perm_oth = perm[1 - rh::2]
    key_idx = np.concatenate([perm_own[:KOWN], perm_oth[:KOWN]])

    mb = np.where(mask[b] != 1, -10000.0, 0.0).astype(f)
    mk = mb[key_idx].reshape(NKY, 128).T

    def wT16(w):
        return _fp8(np.ascontiguousarray(w) * WS)

    def bT(bias, n, scale=1.0):
        return np.ascontiguousarray(bias.reshape(n, 128).T).astype(f) * scale

    # v-bias folds into proj bias exactly: (attn+bv) @ Wp + bp
    bpj_sa = (sa_proj_b + sa_qkv_b[2 * C:] @ sa_proj_w).astype(f)
    bpj_ca = (ca_proj_b + ca_v_b @ ca_proj_w).astype(f)
    return {
        "x_own": np.ascontiguousarray(x[b][perm_own]).astype(f),
        "x_othk": np.ascontiguousarray(x[b][perm_oth[:KOWN]]).astype(f),
        "cT": _fp8(np.ascontiguousarray(c[b].T[:, key_idx])),
        "mask": np.ascontiguousarray(mk).astype(f),
        "wq": wT16(sa_qkv_w[:, 0:C]), "wk": wT16(sa_qkv_w[:, C:2 * C]),
        "wv": wT16(sa_qkv_w[:, 2 * C:3 * C]),
        "wproj": wT16(sa_proj_w),
        "wcaq": wT16(ca_q_w), "wcak": wT16(ca_k_w),
        "wcav": wT16(ca_v_w), "wcaproj": wT16(ca_proj_w),
        "wfc1": wT16(fc1_w), "wfc2": wT16(fc2_w),
        "wfc1l": _fp8(fc1_w * WS - wT16(fc1_w).astype(np.float32)),
        "wfc2l": _fp8(fc2_w * WS - wT16(fc2_w).astype(np.float32)),
        "bq16": bT(sa_qkv_b[0:C], 8, WS),
        "bk16": bT(sa_qkv_b[C:2 * C], 8, WS),
        "bcaq16": bT(ca_q_b, 8, WS),
        "bcak16": bT(ca_k_b, 8, WS),
        "bfc1": bT(fc1_b, 32),
        "bpj": np.concatenate([bpj_sa, bpj_ca, fc2_b.astype(f)]).reshape(1, 3 * C),
        "_perm_own": perm_own,
    }


def kernel(**inputs):
    inputs = {k: np.asarray(v) for k, v in inputs.items()}
    nb = bool(np.any(inputs["sa_proj_b"]) or np.any(inputs["ca_proj_b"])
              or np.any(inputs["fc2_b"]) or np.any(inputs["sa_qkv_b"][2 * C:])
              or np.any(inputs["ca_v_b"]) or np.any(inputs["fc1_b"]))
    key = ("prog", nb)
    if key not in _cache:
        _cache[key] = build_program(nb)
    nc = _cache[key]
    in_maps = [_prep_inputs(i, **inputs) for i in range(8)]
    perms = [m.pop("_perm_own") for m in in_maps]
    res = run_bass_kernel_spmd(nc, in_maps, core_ids=list(range(8)))
    out = np.empty((B, N, C), np.float32)
    for i in range(8):
        b = i // 2
        out[b, perms[i], :] = res.results[i]["y"]
    return out
